# revision 1
# baseline (speedup 1.0000x reference)
"""OHEM cross-entropy loss kernel for Trainium2 (8 NeuronCores, Bass/Tile).

Math (matches reference.py):
    logp   = log_softmax(seg_logit, axis=1)          # [B,C,H,W], C=19
    x_l    = logp at label (ignore 255 -> class 0)
    prob   = exp(x_l)
    thr    = max(sort(prob.flatten())[MIN_KEPT*B], 0.7)
    loss   = mean(-x_l * (prob < thr))

Device strategy (data-parallel over B across 8 cores, one image per core):
    For each pixel p:  t = x_raw[label] - log(sum_c exp(x_raw[c]))  (= logp at label)
    w = 1[t < log(0.7)]   (valid when count(prob<0.7) > MIN_KEPT*B, which the
                           host verifies from the returned counts; otherwise a
                           host fallback computes the exact quantile path)
    Per-core partial sums of (t - log .7)*w (via min(u,0)) and of w are
    returned as [128, 16] partials; host combines:
        sum(-t*w) = -(sum_min + log(.7)*count)

    On-chip per 128x512-pixel chunk:
      - one fat DMA loads [128, 19, 512] f32 logits (class-major)
      - ACT: 19x exp -> bf16 expbuf; pairwise bulk adds (DVE 2x bf16) -> sumexp
      - label gather: 18 in-place copy_predicated mux-tree merges keyed on
        host-provided label bit-planes -> x_l in slot 0
      - ACT: lse = Ln(sumexp); DVE: u = (x_l - log.7) - lse;
        tensor_scalar accum_out reductions of min(u,0) and 1[u<0]
"""

import numpy as np

B = 8
C = 19
H, W = 512, 1024
HW = H * W            # 524288 pixels per image/core
P = 128               # SBUF partitions
FREE = HW // P        # 4096 pixels per partition
F = 512               # chunk free size
NCHUNK = FREE // F    # 8
NBITS = 5             # ceil(log2(19))
C0 = float(np.log(np.float32(0.7)))
MIN_KEPT = 100000
IGNORE_INDEX = 255
N_TOTAL = B * HW

_CACHE = {}


def _build_nc():
    import concourse.bacc as bacc
    import concourse.mybir as mybir
    import concourse.tile as tile

    fp32 = mybir.dt.float32
    bf16 = mybir.dt.bfloat16
    u8 = mybir.dt.uint8

    # Bacc (not plain Bass): its compile pass splits multi-sem sync waits,
    # which the mux-tree copy_predicated instructions need.
    nc = bacc.Bacc()
    logit = nc.dram_tensor("logit", [C, HW], fp32, kind="ExternalInput")
    bits = nc.dram_tensor("bits", [NBITS, P, FREE], u8, kind="ExternalInput")
    acc = nc.dram_tensor("acc", [P, 2 * NCHUNK], fp32, kind="ExternalOutput")

    # [C, (P FREE)] -> [P, C, FREE] view for chunked class-major loads
    logit_v = logit[:, :].rearrange("c (p f) -> p c f", p=P)

    with tile.TileContext(nc) as tc:
        with (
            tc.tile_pool(name="lb", bufs=2) as lb_pool,
            tc.tile_pool(name="eb", bufs=1) as eb_pool,
            tc.tile_pool(name="bits", bufs=1) as bits_pool,
            tc.tile_pool(name="pix", bufs=2) as pix_pool,
            tc.tile_pool(name="accp", bufs=1) as acc_pool,
        ):
            acc_t = acc_pool.tile([P, 2 * NCHUNK], fp32)
            bits_t = bits_pool.tile([P, NBITS, FREE], u8)
            # all 5 bit-planes in one DMA: [NBITS, P, FREE] -> [P, NBITS, FREE]
            nc.sync.dma_start(
                out=bits_t[:], in_=bits[:, :, :].rearrange("k p f -> p k f")
            )

            for j in range(NCHUNK):
                lb = lb_pool.tile([P, C, F], fp32, tag="lb")
                nc.sync.dma_start(out=lb[:], in_=logit_v[:, :, j * F : (j + 1) * F])

                eb = eb_pool.tile([P, C, F], bf16, tag="eb")
                for c in range(C):
                    nc.scalar.activation(
                        out=eb[:, c, :],
                        in_=lb[:, c, :],
                        func=mybir.ActivationFunctionType.Exp,
                    )

                # sumexp: pairwise bulk adds in bf16 (2x mode), final add in f32
                # tree: [0:9]+=[9:18]; [0:4]+=[4:8]; [8]+=[18]; [0:2]+=[2:4];
                #       [0]+=[1]; sumexp = [0]+[8] (f32 out)
                nc.vector.tensor_tensor(
                    out=eb[:, 0:9, :], in0=eb[:, 0:9, :], in1=eb[:, 9:18, :],
                    op=mybir.AluOpType.add,
                )
                nc.vector.tensor_tensor(
                    out=eb[:, 0:4, :], in0=eb[:, 0:4, :], in1=eb[:, 4:8, :],
                    op=mybir.AluOpType.add,
                )
                nc.vector.tensor_tensor(
                    out=eb[:, 8, :], in0=eb[:, 8, :], in1=eb[:, 18, :],
                    op=mybir.AluOpType.add,
                )
                nc.vector.tensor_tensor(
                    out=eb[:, 0:2, :], in0=eb[:, 0:2, :], in1=eb[:, 2:4, :],
                    op=mybir.AluOpType.add,
                )
                nc.vector.tensor_tensor(
                    out=eb[:, 0, :], in0=eb[:, 0, :], in1=eb[:, 1, :],
                    op=mybir.AluOpType.add,
                )
                sumexp = pix_pool.tile([P, F], fp32, tag="sumexp")
                nc.vector.tensor_tensor(
                    out=sumexp[:], in0=eb[:, 0, :], in1=eb[:, 8, :],
                    op=mybir.AluOpType.add,
                )

                lse = pix_pool.tile([P, F], fp32, tag="lse")
                nc.scalar.activation(
                    out=lse[:], in_=sumexp[:], func=mybir.ActivationFunctionType.Ln
                )

                # label mux-tree gather, in place on lb; merge (a, b, bit):
                # lb[:,a,:] <- lb[:,b,:] where bit-plane set
                merges = [
                    *[(2 * i, 2 * i + 1, 0) for i in range(9)],     # bit 0
                    (0, 2, 1), (4, 6, 1), (8, 10, 1), (12, 14, 1), (16, 18, 1),
                    (0, 4, 2), (8, 12, 2),                          # bit 2
                    (0, 8, 3),                                      # bit 3
                    (0, 16, 4),                                     # bit 4
                ]
                bslice = bits_t[:, :, j * F : (j + 1) * F]
                for a, b, k in merges:
                    nc.vector.copy_predicated(
                        out=lb[:, a, :], mask=bslice[:, k, :], data=lb[:, b, :]
                    )

                # u = (x_l - log0.7) - lse; partials: sum(min(u,0)), count(u<0)
                u = pix_pool.tile([P, F], fp32, tag="u")
                nc.vector.scalar_tensor_tensor(
                    out=u[:], in0=lb[:, 0, :], scalar=C0, in1=lse[:],
                    op0=mybir.AluOpType.subtract, op1=mybir.AluOpType.subtract,
                )
                # with accum_out, op1 is the reduce op: accum = reduce(out, op1)
                scr = pix_pool.tile([P, F], fp32, tag="scr")
                nc.vector.tensor_scalar(
                    out=scr[:], in0=u[:], scalar1=0.0, scalar2=None,
                    op0=mybir.AluOpType.min, op1=mybir.AluOpType.add,
                    accum_out=acc_t[:, j : j + 1],
                )
                scr2 = pix_pool.tile([P, F], fp32, tag="scr2")
                nc.vector.tensor_scalar(
                    out=scr2[:], in0=u[:], scalar1=0.0, scalar2=None,
                    op0=mybir.AluOpType.is_lt, op1=mybir.AluOpType.add,
                    accum_out=acc_t[:, NCHUNK + j : NCHUNK + j + 1],
                )

            nc.sync.dma_start(out=acc[:, :], in_=acc_t[:])
    nc.finalize()  # Bacc: runs compile() (reg alloc, sync-wait splitting)
    return nc


def _host_fallback(seg_logit, seg_label):
    """Exact numpy replication of the reference (quantile path included)."""
    x = np.asarray(seg_logit, dtype=np.float32)
    lbl = np.asarray(seg_label)
    Bn, Cn = x.shape[0], x.shape[1]
    xf = x.reshape(Bn, Cn, -1)
    m = xf.max(axis=1, keepdims=True)
    e = np.exp(xf - m)
    lse = np.log(e.sum(axis=1, keepdims=True)) + m
    logp = xf - lse
    l2 = np.where(lbl == IGNORE_INDEX, 0, lbl).reshape(Bn, 1, -1).astype(np.int64)
    lp_at = np.take_along_axis(logp, l2, axis=1)[:, 0]
    prob = np.exp(lp_at)
    sortp = np.sort(prob.reshape(-1))
    idx = min(MIN_KEPT * Bn, sortp.shape[0] - 1)
    thr = max(float(sortp[idx]), np.float32(0.7))
    wgt = (prob < thr).astype(np.float32)
    return np.float32((-lp_at * wgt).mean())


def kernel(seg_logit, seg_label):
    from concourse import bass_utils

    x = np.ascontiguousarray(np.asarray(seg_logit, dtype=np.float32)).reshape(
        B, C, HW
    )
    lbl = np.asarray(seg_label)
    lbl = np.where(lbl == IGNORE_INDEX, 0, lbl).astype(np.uint8).reshape(B, P, FREE)
    # 5 bit-planes per core: [NBITS, P, FREE] uint8
    bits = np.stack(
        [((lbl >> k) & 1).astype(np.uint8) for k in range(NBITS)], axis=1
    )  # [B, NBITS, P, FREE]

    if "nc" not in _CACHE:
        _CACHE["nc"] = _build_nc()
    nc = _CACHE["nc"]

    in_maps = [{"logit": x[b], "bits": bits[b]} for b in range(B)]
    res = bass_utils.run_bass_kernel_spmd(nc, in_maps, core_ids=list(range(B)))

    racc = 0.0
    wacc = 0.0
    for r in res.results:
        a = r["acc"]
        racc += float(a[:, :NCHUNK].sum(dtype=np.float64))
        wacc += float(a[:, NCHUNK:].sum(dtype=np.float64))

    if wacc <= MIN_KEPT * B:
        # quantile threshold exceeds 0.7 -> exact host path (rare/never for
        # the target distribution)
        return _host_fallback(seg_logit, seg_label)

    total = -(racc + C0 * wacc)
    return np.float32(total / N_TOTAL)

